# revision 6
# baseline (speedup 1.0000x reference)
"""Trainium2 Bass kernel for nn_CriticTanh (gnn_message_passing).

Math notes (vs the reference):
  * K=20 == N-1 and the self-distance is +1e10, so the kNN neighbour set of
    node i is exactly {j != i}; max over neighbours is order-independent, so
    the whole pairwise-distance / top_k / gather pipeline is dead code.
  * EdgeConv layer 1 factorizes: [x_i || x_j - x_i] @ mW1
        = x_i @ (mW1[:D] - mW1[D:]) + x_j @ mW1[D:]  =: a_i + b_j
  * The category-embedding path is an affine function of a 3-entry table, so
    tanh(ce) and its contribution to a/b folds into 3-column tables gathered
    via a one-hot matmul (K=3).
  * mb2 commutes with the max, so it becomes the bias of the post-max tanh.

Sharding: data-parallel over batch, 128 samples per core, 8 cores.
"""

import sys

for _p in ("/opt/trn_rl_repo",):
    if _p not in sys.path:
        sys.path.insert(0, _p)

import numpy as np

import concourse.bass as bass
import concourse.bacc as bacc
import concourse.mybir as mybir
from concourse.tile import TileContext
from concourse.bass_utils import run_bass_kernel_spmd

BS, N, K = 1024, 21, 20
HID, EMB, NCAT = 128, 64, 3
D = HID + EMB  # 192
NCORES = 8
S = BS // NCORES          # 128 samples per core
NN = S * N                # 2688 nodes per core
F32 = mybir.dt.float32
BF16 = mybir.dt.float16  # 16-bit compute dtype (fp16: 10-bit mantissa)

# node-stage strips (21-divisible so sample boundaries align)
STRIPS = [(o, min(504, NN - o)) for o in range(0, NN, 504)]
CH = 8                    # samples per edge-stage chunk
NCHUNK = S // CH


def _build_nc():
    nc = bacc.Bacc()

    spatial = nc.declare_dram_parameter("spatial", [4, NN], F32, isOutput=False)
    tar = nc.declare_dram_parameter("tar", [2, NN], F32, isOutput=False)
    onehot = nc.declare_dram_parameter("onehot", [NCAT, NN], F32, isOutput=False)

    wnames = {}
    for h in range(2):
        for name, shape in [
            ("sW1a", [4, HID]), ("sW1b", [2, HID]), ("sb1", [HID, 1]),
            ("sW2", [HID, HID]), ("sb2", [HID, 1]),
            ("Wa1", [HID, HID]), ("Wb1", [HID, HID]),
            ("A2", [NCAT, HID]), ("B2", [NCAT, HID]),
            ("mb1", [HID, 1]), ("mW2", [HID, HID]), ("mb2", [HID, 1]),
            ("tW1", [HID, HID]), ("tb1", [HID, 1]),
            ("tW2", [HID, 1]), ("tb2", [1, 1]),
        ]:
            wnames[(h, name)] = nc.declare_dram_parameter(
                f"{name}_{h}", shape, F32, isOutput=False)

    q_out = [nc.declare_dram_parameter(f"q_{h}", [1, NN], F32, isOutput=True)
             for h in range(2)]

    with TileContext(nc) as tc:
        with (
            tc.tile_pool(name="persist", bufs=1) as pp,
            tc.tile_pool(name="work", bufs=3) as wp,
            tc.tile_pool(name="psum", bufs=3, space="PSUM") as ps,
        ):
            # ---- load inputs / weights to SBUF ----
            sp_sb = pp.tile([4, NN], F32, tag="spatial", name="spatial")
            nc.sync.dma_start(out=sp_sb[:], in_=spatial[:])
            tar_sb = pp.tile([2, NN], F32, tag="tar", name="tar")
            nc.sync.dma_start(out=tar_sb[:], in_=tar[:])
            oh_sb = pp.tile([NCAT, NN], F32, tag="onehot", name="onehot")
            nc.sync.dma_start(out=oh_sb[:], in_=onehot[:])
            nc.scalar.activation(tar_sb[:], tar_sb[:],
                                 mybir.ActivationFunctionType.Tanh)

            W = {}
            for (h, name), dram in wnames.items():
                t = pp.tile(list(dram.shape), F32, tag=f"{name}_{h}")
                nc.sync.dma_start(out=t[:], in_=dram[:])
                W[(h, name)] = t
            # bf16 copies for the bf16 matmuls
            Wb = {}
            for h in range(2):
                for name in ("mW2", "tW1", "tW2"):
                    src = W[(h, name)]
                    t = pp.tile(list(src.shape), BF16, tag=f"{name}b_{h}")
                    nc.vector.tensor_copy(t[:], src[:])
                    Wb[(h, name)] = t

            TANH = mybir.ActivationFunctionType.Tanh
            IDENT = mybir.ActivationFunctionType.Identity

            a_sb, bd_sb, agp_sb, agg_sb = {}, {}, {}, {}
            for h in range(2):
                a_sb[h] = pp.tile([HID, NN], BF16, tag=f"a_{h}", name=f"a_{h}")
                bd_sb[h] = pp.tile([HID, S, 2 * N], BF16, tag=f"bd_{h}", name=f"bd_{h}")
                agp_sb[h] = pp.tile([HID, NN], BF16, tag=f"agp_{h}", name=f"agp_{h}")
                agg_sb[h] = pp.tile([HID, NN], BF16, tag=f"agg_{h}", name=f"agg_{h}")

            # ================= node stage =================
            for h in range(2):
                htop = pp.tile([HID, NN], F32, tag=f"htop_{h}", name=f"htop_{h}")
                for off, w in STRIPS:
                    p1 = ps.tile([HID, 512], F32, tag="ps", name="ps")
                    nc.tensor.matmul(p1[:, :w], W[(h, "sW1a")][:],
                                     sp_sb[:, off:off + w], start=True, stop=False)
                    nc.tensor.matmul(p1[:, :w], W[(h, "sW1b")][:],
                                     tar_sb[:, off:off + w], start=False, stop=True)
                    h1 = wp.tile([HID, 504], F32, tag="h1", name="h1")
                    nc.scalar.activation(h1[:, :w], p1[:, :w], TANH,
                                         bias=W[(h, "sb1")][:])
                    p2 = ps.tile([HID, 512], F32, tag="ps", name="ps")
                    nc.tensor.matmul(p2[:, :w], W[(h, "sW2")][:], h1[:, :w])
                    nc.scalar.activation(htop[:, off:off + w], p2[:, :w], TANH,
                                         bias=W[(h, "sb2")][:])
                for off, w in STRIPS:
                    s0, ns = off // N, w // N
                    pa = ps.tile([HID, 512], F32, tag="ps", name="ps")
                    nc.tensor.matmul(pa[:, :w], W[(h, "Wa1")][:],
                                     htop[:, off:off + w], start=True, stop=False)
                    nc.tensor.matmul(pa[:, :w], W[(h, "A2")][:],
                                     oh_sb[:, off:off + w], start=False, stop=True)
                    nc.vector.tensor_scalar_add(a_sb[h][:, off:off + w],
                                                pa[:, :w], W[(h, "mb1")][:])
                    pb = ps.tile([HID, 512], F32, tag="ps", name="ps")
                    nc.tensor.matmul(pb[:, :w], W[(h, "Wb1")][:],
                                     htop[:, off:off + w], start=True, stop=False)
                    nc.tensor.matmul(pb[:, :w], W[(h, "B2")][:],
                                     oh_sb[:, off:off + w], start=False, stop=True)
                    nc.vector.tensor_copy(bd_sb[h][:, s0:s0 + ns, 0:N], pb[:, :w])
                    nc.vector.tensor_copy(bd_sb[h][:, s0:s0 + ns, N:2 * N],
                                          bd_sb[h][:, s0:s0 + ns, 0:N])

            # ================= edge stage =================
            for h in range(2):
                for c in range(NCHUNK):
                    s0 = c * CH
                    # T_pre[p, s, jj, i] = a[p, s, i] + b[p, s, 1 + jj + i]
                    a_bc = (a_sb[h][:, s0 * N:(s0 + CH) * N]
                            .rearrange("p (s i) -> p s i", s=CH)
                            .unsqueeze(2)
                            .broadcast_to([HID, CH, K, N]))
                    bsl = bd_sb[h][:, s0:s0 + CH, 1:1 + N]
                    b_win = bass.AP(tensor=bsl.tensor, offset=bsl.offset,
                                    ap=[list(bsl.ap[0]), [2 * N, CH],
                                        [1, K], [1, N]])
                    tp = wp.tile([HID, CH, K, N], BF16, tag="tp", name="tp")
                    nc.vector.tensor_add(tp[:], a_bc, b_win)
                    tt = wp.tile([HID, CH, K, N], BF16, tag="tt", name="tt")
                    nc.scalar.activation(tt[:], tp[:], TANH)
                    for g in range(4):
                        mp = ps.tile([HID, 2, 512], F32, tag="mp", name="mp",
                                     bufs=2)
                        for k in range(2):
                            s = g * 2 + k
                            nc.tensor.matmul(mp[:, k, 0:K * N], Wb[(h, "mW2")][:],
                                             tt[:, s])
                        red_in = mp[:, :, 0:K * N].rearrange(
                            "p s (jj i) -> p s i jj", jj=K, i=N)
                        o0 = s0 * N + g * 2 * N
                        nc.vector.tensor_reduce(
                            agp_sb[h][:, o0:o0 + 2 * N], red_in,
                            axis=mybir.AxisListType.X, op=mybir.AluOpType.max)
                nc.scalar.activation(agg_sb[h][:], agp_sb[h][:], TANH,
                                     bias=W[(h, "mb2")][:])

            # ================= output head =================
            for h in range(2):
                qsb = pp.tile([1, NN], F32, tag=f"qsb_{h}", name=f"qsb_{h}")
                for off, w in STRIPS:
                    pz = ps.tile([HID, 512], F32, tag="ps", name="ps")
                    nc.tensor.matmul(pz[:, :w], Wb[(h, "tW1")][:],
                                     agg_sb[h][:, off:off + w])
                    z = wp.tile([HID, 504], BF16, tag="z", name="z")
                    nc.scalar.activation(z[:, :w], pz[:, :w], TANH,
                                         bias=W[(h, "tb1")][:])
                    pq = ps.tile([1, 512], F32, tag="psq", name="psq", bufs=1)
                    nc.tensor.matmul(pq[:, :w], Wb[(h, "tW2")][:], z[:, :w])
                    nc.scalar.activation(qsb[:, off:off + w], pq[:, :w], IDENT,
                                         bias=W[(h, "tb2")][:])
                nc.sync.dma_start(out=q_out[h][:], in_=qsb[:])

    nc.compile()
    return nc


_NC_CACHE = None


def _get_nc():
    global _NC_CACHE
    if _NC_CACHE is None:
        _NC_CACHE = _build_nc()
    return _NC_CACHE


def _head_consts(p):
    p = {k: np.asarray(v, np.float32) for k, v in p.items()}
    Wa = p["mW1"][:D] - p["mW1"][D:]
    Wb = p["mW1"][D:]
    hce = np.tanh(np.tanh(p["emb"]) @ p["eW"] + p["eb"])          # [3, EMB]
    out = {
        "sW1a": p["sW1"][0:4], "sW1b": p["sW1"][4:6],
        "sb1": p["sb1"].reshape(HID, 1),
        "sW2": p["sW2"], "sb2": p["sb2"].reshape(HID, 1),
        "Wa1": Wa[:HID], "Wb1": Wb[:HID],
        "A2": hce @ Wa[HID:], "B2": hce @ Wb[HID:],
        "mb1": p["mb1"].reshape(HID, 1), "mW2": p["mW2"],
        "mb2": p["mb2"].reshape(HID, 1),
        "tW1": p["tW1"], "tb1": p["tb1"].reshape(HID, 1),
        "tW2": p["tW2"], "tb2": p["tb2"].reshape(1, 1),
    }
    return {k: np.ascontiguousarray(v, np.float32) for k, v in out.items()}


def kernel(positions, categories, action, tar_scores, params):
    positions = np.asarray(positions, np.float32)
    categories = np.asarray(categories)
    action = np.asarray(action, np.float32)
    tar_scores = np.asarray(tar_scores, np.float32)

    nc = _get_nc()

    wmaps = {}
    for h, key in enumerate(("h1", "h2")):
        for name, arr in _head_consts(params[key]).items():
            wmaps[f"{name}_{h}"] = arr

    in_maps = []
    for c in range(NCORES):
        r0 = c * NN
        rows = slice(r0, r0 + NN)
        spat = np.empty((4, NN), np.float32)
        spat[0:2] = positions[rows].T
        spat[2:4] = action[c * S:(c + 1) * S].reshape(NN, 2).T
        oh = (categories[rows][None, :] == np.arange(NCAT)[:, None])
        in_maps.append({"spatial": spat,
                        "tar": np.ascontiguousarray(tar_scores[rows].T),
                        "onehot": np.ascontiguousarray(oh, np.float32),
                        **wmaps})

    res = run_bass_kernel_spmd(nc, in_maps, core_ids=list(range(NCORES)))
    qs = []
    for h in range(2):
        q = np.concatenate(
            [res.results[c][f"q_{h}"].reshape(S, N) for c in range(NCORES)], 0)
        qs.append(np.ascontiguousarray(q, np.float32))
    return tuple(qs)


if __name__ == "__main__":
    rng = np.random.default_rng(0)
    inputs = {
        "positions": rng.standard_normal((BS * N, 2), np.float32),
        "categories": rng.integers(0, NCAT, BS * N).astype(np.int32),
        "action": rng.standard_normal((BS, N * 2), np.float32),
        "tar_scores": rng.standard_normal((BS * N, 2), np.float32),
        "params": {
            k: {
                "sW1": rng.standard_normal((6, HID), np.float32),
                "sb1": np.zeros(HID, np.float32),
                "sW2": rng.standard_normal((HID, HID), np.float32),
                "sb2": np.zeros(HID, np.float32),
                "emb": rng.standard_normal((NCAT, EMB), np.float32),
                "eW": rng.standard_normal((EMB, EMB), np.float32),
                "eb": np.zeros(EMB, np.float32),
                "mW1": rng.standard_normal((2 * D, HID), np.float32),
                "mb1": np.zeros(HID, np.float32),
                "mW2": rng.standard_normal((HID, HID), np.float32),
                "mb2": np.zeros(HID, np.float32),
                "tW1": rng.standard_normal((HID, HID), np.float32),
                "tb1": np.zeros(HID, np.float32),
                "tW2": rng.standard_normal((HID, 1), np.float32),
                "tb2": np.zeros(1, np.float32),
            } for k in ("h1", "h2")
        },
    }
    q1, q2 = kernel(**inputs)
    print(q1.shape, q2.shape, q1[:2, :4])
